# revision 30
# baseline (speedup 1.0000x reference)
"""Top-k gated mixture of linear maps (MoE routing) on 8 TRN2 NeuronCores.

Expert-parallel sharding: the 16 charts are assigned 2-per-core (balanced
pairing: largest chart with smallest). Routing (top-2 chart selection +
gate normalization, an 8192x16 argmax — 0.003% of the FLOPs) runs on host
as part of the dispatch/sharding step; each core receives the tokens
routed to its two charts as transposed column blocks plus per-token gate
scales, with its two charts' residual weights resident in SBUF.

Residual decomposition: with gates g1+g2 = s, algebraically
    out = g1*W_c1 q + g2*W_c2 q = g1*(W_c1+I) q + g2*(W_c2+I) q - s*q.
The device computes the (W+I) matmuls in bf16 — since W ~= -I + 0.01*N,
the residual W+I is ~100x smaller than q's projection, so bf16 rounding
error scales with the residual, not the output (measured ~6e-4 scale-
relative absmax vs fp32 reference). Accumulation is fp32 in PSUM, gate
scaling on-device in fp32, and the exact -s*q correction is applied in
fp32 during the host unshard/combine step.

Matmul orientation: residual W^T chunks are the stationary operand and
token columns are the moving operand (out = Y^T tiles [u, tokens]), so
token capacities need no 128-alignment — per-slot capacity is exactly
the max chart load and the only padded work is capacity skew.

self-contained: only imports concourse (globally installed) + numpy.
"""

import numpy as np

P = 128
D = 1024  # DIM_Q == DIM_U
NUM_CHARTS = 16
TOP_K = 2
N_CORES = 8
MAX_MM = 512  # moving-operand / PSUM-bank limit per matmul

_NC_CACHE: dict = {}


def _blocks(cap0: int, cap1: int):
    """Static token-block list: (slot, col0, length) covering [0, cap0+cap1),
    slot-contiguous, each block <= MAX_MM, evenly split per slot."""
    out = []
    col = 0
    for s, cap in ((0, cap0), (1, cap1)):
        nb = -(-cap // MAX_MM)
        base, extra = divmod(cap, nb)
        for b in range(nb):
            ln = base + (1 if b < extra else 0)
            out.append((s, col, ln))
            col += ln
    return out


def _build(cap0: int, cap1: int):
    """Build + compile the SPMD single-core program (same on all 8 cores).

    Inputs per core (CT = cap0 + cap1):
      xt  [8, 128, CT] bf16 : X^T q-chunked: xt[qc, p, col] =
                              q[token(col)][qc*128+p]. Slot 0 tokens in
                              columns [0, cap0), slot 1 after (zero-pad).
      wt  [2, 8, 128, 1024] bf16 : (W+I)^T of the 2 charts, q-chunked the
                              same way: wt[s, qc, p, u].
      scv [1, CT] f32 : gate scale of token(col); broadcast across
                              partitions on device via k=1 matmuls.
    Output:
      y [1024, CT] f32 : Y^T — gate-scaled residual chart outputs.
    """
    import concourse.mybir as mybir
    from concourse import bacc
    from concourse.tile import TileContext

    CT = cap0 + cap1
    f32 = mybir.dt.float32
    bf16 = mybir.dt.bfloat16
    blocks = _blocks(cap0, cap1)

    nc = bacc.Bacc("TRN2", target_bir_lowering=False, debug=False)
    xt = nc.dram_tensor("xt", [8, P, CT], bf16, kind="ExternalInput")
    wt = nc.dram_tensor("wt", [2, 8, P, D], bf16, kind="ExternalInput")
    scv = nc.dram_tensor("scv", [1, CT], f32, kind="ExternalInput")
    y = nc.dram_tensor("y", [D, CT], f32, kind="ExternalOutput")

    with TileContext(nc) as tc:
        with (
            tc.tile_pool(name="wpool", bufs=1) as wpool,
            tc.tile_pool(name="xpool", bufs=1) as xpool,
            tc.tile_pool(name="spool", bufs=1) as spool,
            tc.tile_pool(name="opool", bufs=3) as opool,
            tc.tile_pool(name="psum", bufs=8, space="PSUM") as psum,
        ):
            # gate-scale broadcast [1, CT] -> [128, CT] via k=1 matmuls with
            # a ones column; cheap, and warms the PE during the preload.
            sc_row = spool.tile([1, CT], f32, name="sc_row")
            nc.scalar.dma_start(sc_row[:], scv[:])
            ones = spool.tile([1, P], f32, name="ones")
            nc.vector.memset(ones[:], 1.0)
            sc_sb = spool.tile([P, CT], f32, name="sc_sb")
            for bi, (_s, c0, ln) in enumerate(blocks):
                bps = psum.tile([P, ln], f32, tag="ps", name=f"bps_{bi}")
                nc.tensor.matmul(
                    bps[:],
                    lhsT=ones[:],
                    rhs=sc_row[:, c0 : c0 + ln],
                    start=True,
                    stop=True,
                )
                nc.vector.tensor_copy(sc_sb[:, c0 : c0 + ln], bps[:])

            # X^T resident: 8 q-chunk tiles [128, CT]; residual W^T resident:
            # 8 per-q-chunk tiles per slot (fine-grained dependencies). Issue
            # order: (w0,x,w1) per q-chunk so accumulation over qc can chase
            # the arrival front.
            x_sb = [None] * 8
            w_sb = {0: [None] * 8, 1: [None] * 8}

            def _load_x(qc):
                t = xpool.tile([P, CT], bf16, tag=f"x_{qc}", name=f"x_{qc}")
                nc.sync.dma_start(t[:], xt[qc])
                x_sb[qc] = t

            def _load_w(s, qc):
                t = wpool.tile([P, D], bf16, tag=f"w_{s}_{qc}", name=f"w_{s}_{qc}")
                nc.sync.dma_start(t[:], wt[s, qc])
                w_sb[s][qc] = t

            for qc in range(8):
                _load_w(0, qc)
                _load_x(qc)
                _load_w(1, qc)

            for u in range(8):
                pss = [
                    psum.tile([P, ln], f32, tag="ps", name=f"ps_{u}_{bi}")
                    for bi, (_s, _c, ln) in enumerate(blocks)
                ]
                # qc outer, blocks inner: the stationary operand changes only
                # twice per qc (once per slot), and accumulation can chase the
                # X-chunk arrival front during the preload. The final column
                # runs block-outer instead so the tail evictions pipeline with
                # the remaining matmuls.
                if u < 7:
                    mm_order = [(qc, bi) for qc in range(8) for bi in range(len(blocks))]
                else:
                    mm_order = [(qc, bi) for bi in range(len(blocks)) for qc in range(8)]
                for qc, bi in mm_order:
                    s, c0, ln = blocks[bi]
                    nc.tensor.matmul(
                        pss[bi][:],
                        lhsT=w_sb[s][qc][:, u * P : (u + 1) * P],
                        rhs=x_sb[qc][:, c0 : c0 + ln],
                        start=(qc == 0),
                        stop=(qc == 7),
                    )
                # per-block eviction into a shared row tile; one Y^T DMA per
                # slot half (ACT queue, so the waits don't head-of-line-block
                # the loads on SP). The final column stores per block so the
                # tail DMA pipelines with the last evictions.
                ot = opool.tile([P, CT], f32, tag="o", name=f"o_{u}")
                prev_s = 0
                for bi, (s, c0, ln) in enumerate(blocks):
                    if u < 7 and s != prev_s:
                        nc.scalar.dma_start(
                            y[u * P : (u + 1) * P, 0:cap0], ot[:, 0:cap0]
                        )
                        prev_s = s
                    nc.vector.tensor_tensor(
                        out=ot[:, c0 : c0 + ln],
                        in0=pss[bi][:],
                        in1=sc_sb[:, c0 : c0 + ln],
                        op=mybir.AluOpType.mult,
                    )
                    if u == 7:
                        nc.scalar.dma_start(
                            y[u * P : (u + 1) * P, c0 : c0 + ln], ot[:, c0 : c0 + ln]
                        )
                if u < 7:
                    lo = cap0 if prev_s == 1 else 0
                    nc.scalar.dma_start(
                        y[u * P : (u + 1) * P, lo:CT], ot[:, lo:CT]
                    )
    nc.compile()
    return nc


def _get_nc(cap0: int, cap1: int):
    key = (cap0, cap1)
    if key not in _NC_CACHE:
        _NC_CACHE[key] = _build(cap0, cap1)
    return _NC_CACHE[key]


def _route(weights: np.ndarray):
    """Host router: top-2 charts + normalized gates (matches jax.lax.top_k
    tie-breaking: lower index first)."""
    b = weights.shape[0]
    ar = np.arange(b)
    i1 = np.argmax(weights, axis=1)
    v1 = weights[ar, i1]
    w2 = weights.copy()
    w2[ar, i1] = -np.inf
    i2 = np.argmax(w2, axis=1)
    v2 = w2[ar, i2]
    s = np.clip(v1 + v2, 1e-8, None)
    return i1, i2, v1 / s, v2 / s


def kernel(q, weights, W_stack):
    import ml_dtypes

    q = np.ascontiguousarray(np.asarray(q, dtype=np.float32))
    weights = np.ascontiguousarray(np.asarray(weights, dtype=np.float32))
    W_stack = np.asarray(W_stack, dtype=np.float32)

    B = q.shape[0]
    i1, i2, g1, g2 = _route(weights)

    # flatten the (token, chart) pairs and group by chart
    flat_chart = np.concatenate([i1, i2])
    flat_tok = np.concatenate([np.arange(B), np.arange(B)])
    flat_gate = np.concatenate([g1, g2]).astype(np.float32)
    order = np.argsort(flat_chart, kind="stable")
    counts = np.bincount(flat_chart, minlength=NUM_CHARTS)
    starts = np.zeros(NUM_CHARTS + 1, dtype=np.int64)
    starts[1:] = np.cumsum(counts)

    # balanced pairing: largest chart with smallest on the same core
    by_size = np.argsort(-counts, kind="stable")
    slot_chart = np.empty((N_CORES, 2), dtype=np.int64)
    for core in range(N_CORES):
        slot_chart[core, 0] = by_size[core]
        slot_chart[core, 1] = by_size[NUM_CHARTS - 1 - core]

    cap0 = int(counts[slot_chart[:, 0]].max())
    cap1 = int(counts[slot_chart[:, 1]].max())
    CT = cap0 + cap1

    # residual weights (W + I)^T per chart, bf16
    eye = np.eye(D, dtype=np.float32)
    Rt = (W_stack.transpose(0, 2, 1) + eye[None]).astype(ml_dtypes.bfloat16)
    q_bf = q.astype(ml_dtypes.bfloat16)

    in_maps = []
    pos = np.empty(2 * B, dtype=np.int64)  # pair j -> global Y^T column
    for core in range(N_CORES):
        xp = np.zeros((CT, D), dtype=ml_dtypes.bfloat16)  # packed tokens
        scv_c = np.zeros(CT, dtype=np.float32)
        wtm = np.empty((2, 8, P, D), dtype=ml_dtypes.bfloat16)
        for slot in range(2):
            c = int(slot_chart[core, slot])
            wtm[slot] = Rt[c].reshape(8, P, D)
            sel = order[starts[c] : starts[c + 1]]
            n = len(sel)
            col0 = slot * cap0
            xp[col0 : col0 + n] = q_bf[flat_tok[sel]]
            scv_c[col0 : col0 + n] = flat_gate[sel]
            pos[sel] = core * CT + col0 + np.arange(n)
        xtm = np.ascontiguousarray(xp.T).reshape(8, P, CT)
        in_maps.append({"xt": xtm, "wt": wtm, "scv": scv_c[None, :]})

    from concourse.bass_utils import run_bass_kernel_spmd

    nc = _get_nc(cap0, cap1)
    res = run_bass_kernel_spmd(nc, in_maps, core_ids=list(range(N_CORES)))
    # y is Y^T [1024, CT] per core; stack as [8*CT, 1024] token-major
    Yg = np.concatenate(
        [np.ascontiguousarray(np.asarray(res.results[i]["y"]).T) for i in range(N_CORES)],
        axis=0,
    )

    # exact combine: out = g1*Y1 + g2*Y2 - (g1+g2)*q   (Y are residual outs)
    gs = (g1 + g2).astype(np.float32)
    out = Yg[pos[:B]] + Yg[pos[B:]] - gs[:, None] * q
    return np.ascontiguousarray(out, dtype=np.float32)


# revision 37
# speedup vs baseline: 1.0119x; 1.0119x over previous
"""Top-k gated mixture of linear maps (MoE routing) on 8 TRN2 NeuronCores.

Expert-parallel sharding: the 16 charts are assigned 2-per-core (balanced
pairing: largest chart with smallest). Routing (top-2 chart selection +
gate normalization, an 8192x16 argmax — 0.003% of the FLOPs) runs on host
as part of the dispatch/sharding step; each core receives the tokens
routed to its two charts as transposed column blocks plus per-token gate
scales, with its two charts' residual weights resident in SBUF.

Residual decomposition: with gates g1+g2 = s, algebraically
    out = g1*W_c1 q + g2*W_c2 q = g1*(W_c1+I) q + g2*(W_c2+I) q - s*q.
The device computes the (W+I) matmuls in bf16 — since W ~= -I + 0.01*N,
the residual W+I is ~100x smaller than q's projection, so bf16 rounding
error scales with the residual, not the output (measured ~6e-4 scale-
relative absmax vs fp32 reference). Accumulation is fp32 in PSUM, gate
scaling on-device in fp32, and the exact -s*q correction is applied in
fp32 during the host unshard/combine step.

Matmul orientation: residual W^T chunks are the stationary operand and
token columns are the moving operand (out = Y^T tiles [u, tokens]), so
token capacities need no 128-alignment — per-slot capacity is exactly
the max chart load and the only padded work is capacity skew.

self-contained: only imports concourse (globally installed) + numpy.
"""

import numpy as np

P = 128
D = 1024  # DIM_Q == DIM_U
NUM_CHARTS = 16
TOP_K = 2
N_CORES = 8
MAX_MM = 512  # moving-operand / PSUM-bank limit per matmul

_NC_CACHE: dict = {}


def _blocks(cap0: int, cap1: int):
    """Static token-block list: (slot, col0, length) covering [0, cap0+cap1),
    slot-contiguous, each block <= MAX_MM. Greedy max-size blocks (bigger
    early blocks = more PE work per PSUM bank during the preload), with the
    tail rebalanced to avoid degenerate slivers."""
    out = []
    col = 0
    for s, cap in ((0, cap0), (1, cap1)):
        lens = []
        left = cap
        while left > 0:
            ln = min(MAX_MM, left)
            lens.append(ln)
            left -= ln
        if len(lens) >= 2 and lens[-1] < 64:
            merged = lens[-2] + lens[-1]
            lens[-2], lens[-1] = -(-merged // 2), merged // 2
        for ln in lens:
            out.append((s, col, ln))
            col += ln
    return out


def _build(cap0: int, cap1: int):
    """Build + compile the SPMD single-core program (same on all 8 cores).

    Inputs per core (CT = cap0 + cap1):
      xt  [8, 128, CT] bf16 : X^T q-chunked: xt[qc, p, col] =
                              q[token(col)][qc*128+p]. Slot 0 tokens in
                              columns [0, cap0), slot 1 after (zero-pad).
      wt  [2, 8, 128, 1024] bf16 : (W+I)^T of the 2 charts, q-chunked the
                              same way: wt[s, qc, p, u].
      scv [1, CT] f32 : gate scale of token(col); broadcast across
                              partitions on device via k=1 matmuls.
    Output:
      y [1024, CT] f32 : Y^T — gate-scaled residual chart outputs.
    """
    import concourse.mybir as mybir
    from concourse import bacc
    from concourse.tile import TileContext

    CT = cap0 + cap1
    f32 = mybir.dt.float32
    bf16 = mybir.dt.bfloat16
    blocks = _blocks(cap0, cap1)

    nc = bacc.Bacc("TRN2", target_bir_lowering=False, debug=False)
    xt = nc.dram_tensor("xt", [8, P, CT], bf16, kind="ExternalInput")
    wt = nc.dram_tensor("wt", [2, 8, P, D], bf16, kind="ExternalInput")
    scv = nc.dram_tensor("scv", [1, CT], f32, kind="ExternalInput")
    y = nc.dram_tensor("y", [D, CT], f32, kind="ExternalOutput")

    with TileContext(nc) as tc:
        with (
            tc.tile_pool(name="wpool", bufs=1) as wpool,
            tc.tile_pool(name="xpool", bufs=1) as xpool,
            tc.tile_pool(name="spool", bufs=1) as spool,
            tc.tile_pool(name="opool", bufs=3) as opool,
            tc.tile_pool(name="psum", bufs=8, space="PSUM") as psum,
        ):
            # gate-scale broadcast [1, CT] -> [128, CT] via k=1 matmuls with
            # a ones column; cheap, and warms the PE during the preload.
            # sc_row's DMA is issued first so the broadcast starts ASAP.
            sc_row = spool.tile([1, CT], f32, name="sc_row")
            nc.sync.dma_start(sc_row[:], scv[:])
            ones = spool.tile([1, P], f32, name="ones")
            nc.vector.memset(ones[:], 1.0)
            sc_sb = spool.tile([P, CT], f32, name="sc_sb")
            for bi, (_s, c0, ln) in enumerate(blocks):
                bps = psum.tile([P, ln], f32, tag="ps", name=f"bps_{bi}")
                nc.tensor.matmul(
                    bps[:],
                    lhsT=ones[:],
                    rhs=sc_row[:, c0 : c0 + ln],
                    start=True,
                    stop=True,
                )
                nc.vector.tensor_copy(sc_sb[:, c0 : c0 + ln], bps[:])

            # X^T resident: 8 q-chunk tiles [128, CT]; residual W^T resident:
            # 8 per-q-chunk tiles per slot (fine-grained dependencies). Issue
            # order: (w0,x,w1) per q-chunk so accumulation over qc can chase
            # the arrival front.
            x_sb = [None] * 8
            w_sb = {0: [None] * 8, 1: [None] * 8}

            def _load_x(qc):
                t = xpool.tile([P, CT], bf16, tag=f"x_{qc}", name=f"x_{qc}")
                nc.sync.dma_start(t[:], xt[qc])
                x_sb[qc] = t

            def _load_w(s, qc):
                t = wpool.tile([P, D], bf16, tag=f"w_{s}_{qc}", name=f"w_{s}_{qc}")
                nc.sync.dma_start(t[:], wt[s, qc])
                w_sb[s][qc] = t

            for qc in range(8):
                _load_w(0, qc)
                _load_x(qc)
                _load_w(1, qc)

            for u in range(8):
                pss = [
                    psum.tile([P, ln], f32, tag="ps", name=f"ps_{u}_{bi}")
                    for bi, (_s, _c, ln) in enumerate(blocks)
                ]
                # Early columns run qc-outer/blocks-inner so accumulation can
                # chase the X-chunk arrival front during the preload. Once X
                # is certainly resident (u >= 3), run block-outer: each PSUM
                # group completes in 8 consecutive matmuls, banks recycle
                # fast, and evictions spread evenly instead of bursting.
                if u < 4:
                    mm_order = [(qc, bi) for qc in range(8) for bi in range(len(blocks))]
                else:
                    mm_order = [(qc, bi) for bi in range(len(blocks)) for qc in range(8)]
                for qc, bi in mm_order:
                    s, c0, ln = blocks[bi]
                    nc.tensor.matmul(
                        pss[bi][:],
                        lhsT=w_sb[s][qc][:, u * P : (u + 1) * P],
                        rhs=x_sb[qc][:, c0 : c0 + ln],
                        start=(qc == 0),
                        stop=(qc == 7),
                    )
                # per-block eviction into a shared row tile; one Y^T DMA per
                # slot half (ACT queue, so the waits don't head-of-line-block
                # the loads on SP). The final column stores per block so the
                # tail DMA pipelines with the last evictions.
                ot = opool.tile([P, CT], f32, tag="o", name=f"o_{u}")
                prev_s = 0
                for bi, (s, c0, ln) in enumerate(blocks):
                    if u < 7 and s != prev_s:
                        nc.scalar.dma_start(
                            y[u * P : (u + 1) * P, 0:cap0], ot[:, 0:cap0]
                        )
                        prev_s = s
                    nc.vector.tensor_tensor(
                        out=ot[:, c0 : c0 + ln],
                        in0=pss[bi][:],
                        in1=sc_sb[:, c0 : c0 + ln],
                        op=mybir.AluOpType.mult,
                    )
                    if u == 7:
                        nc.scalar.dma_start(
                            y[u * P : (u + 1) * P, c0 : c0 + ln], ot[:, c0 : c0 + ln]
                        )
                if u < 7:
                    lo = cap0 if prev_s == 1 else 0
                    nc.scalar.dma_start(
                        y[u * P : (u + 1) * P, lo:CT], ot[:, lo:CT]
                    )
    nc.compile()
    return nc


def _get_nc(cap0: int, cap1: int):
    key = (cap0, cap1)
    if key not in _NC_CACHE:
        _NC_CACHE[key] = _build(cap0, cap1)
    return _NC_CACHE[key]


def _route(weights: np.ndarray):
    """Host router: top-2 charts + normalized gates (matches jax.lax.top_k
    tie-breaking: lower index first)."""
    b = weights.shape[0]
    ar = np.arange(b)
    i1 = np.argmax(weights, axis=1)
    v1 = weights[ar, i1]
    w2 = weights.copy()
    w2[ar, i1] = -np.inf
    i2 = np.argmax(w2, axis=1)
    v2 = w2[ar, i2]
    s = np.clip(v1 + v2, 1e-8, None)
    return i1, i2, v1 / s, v2 / s


def kernel(q, weights, W_stack):
    import ml_dtypes

    q = np.ascontiguousarray(np.asarray(q, dtype=np.float32))
    weights = np.ascontiguousarray(np.asarray(weights, dtype=np.float32))
    W_stack = np.asarray(W_stack, dtype=np.float32)

    B = q.shape[0]
    i1, i2, g1, g2 = _route(weights)

    # flatten the (token, chart) pairs and group by chart
    flat_chart = np.concatenate([i1, i2])
    flat_tok = np.concatenate([np.arange(B), np.arange(B)])
    flat_gate = np.concatenate([g1, g2]).astype(np.float32)
    order = np.argsort(flat_chart, kind="stable")
    counts = np.bincount(flat_chart, minlength=NUM_CHARTS)
    starts = np.zeros(NUM_CHARTS + 1, dtype=np.int64)
    starts[1:] = np.cumsum(counts)

    # balanced pairing: largest chart with smallest on the same core
    by_size = np.argsort(-counts, kind="stable")
    slot_chart = np.empty((N_CORES, 2), dtype=np.int64)
    for core in range(N_CORES):
        slot_chart[core, 0] = by_size[core]
        slot_chart[core, 1] = by_size[NUM_CHARTS - 1 - core]

    cap0 = int(counts[slot_chart[:, 0]].max())
    cap1 = int(counts[slot_chart[:, 1]].max())
    CT = cap0 + cap1

    # residual weights (W + I)^T per chart, bf16
    eye = np.eye(D, dtype=np.float32)
    Rt = (W_stack.transpose(0, 2, 1) + eye[None]).astype(ml_dtypes.bfloat16)
    q_bf = q.astype(ml_dtypes.bfloat16)

    in_maps = []
    pos = np.empty(2 * B, dtype=np.int64)  # pair j -> global Y^T column
    for core in range(N_CORES):
        xp = np.zeros((CT, D), dtype=ml_dtypes.bfloat16)  # packed tokens
        scv_c = np.zeros(CT, dtype=np.float32)
        wtm = np.empty((2, 8, P, D), dtype=ml_dtypes.bfloat16)
        for slot in range(2):
            c = int(slot_chart[core, slot])
            wtm[slot] = Rt[c].reshape(8, P, D)
            sel = order[starts[c] : starts[c + 1]]
            n = len(sel)
            col0 = slot * cap0
            xp[col0 : col0 + n] = q_bf[flat_tok[sel]]
            scv_c[col0 : col0 + n] = flat_gate[sel]
            pos[sel] = core * CT + col0 + np.arange(n)
        xtm = np.ascontiguousarray(xp.T).reshape(8, P, CT)
        in_maps.append({"xt": xtm, "wt": wtm, "scv": scv_c[None, :]})

    from concourse.bass_utils import run_bass_kernel_spmd

    nc = _get_nc(cap0, cap1)
    res = run_bass_kernel_spmd(nc, in_maps, core_ids=list(range(N_CORES)))
    # y is Y^T [1024, CT] per core; stack as [8*CT, 1024] token-major
    Yg = np.concatenate(
        [np.ascontiguousarray(np.asarray(res.results[i]["y"]).T) for i in range(N_CORES)],
        axis=0,
    )

    # exact combine: out = g1*Y1 + g2*Y2 - (g1+g2)*q   (Y are residual outs)
    gs = (g1 + g2).astype(np.float32)
    out = Yg[pos[:B]] + Yg[pos[B:]] - gs[:, None] * q
    return np.ascontiguousarray(out, dtype=np.float32)


# revision 42
# speedup vs baseline: 1.0976x; 1.0848x over previous
"""Top-k gated mixture of linear maps (MoE routing) on 8 TRN2 NeuronCores.

Expert-parallel sharding: the 16 charts are assigned 2-per-core (balanced
pairing: largest chart with smallest). Routing (top-2 chart selection +
gate normalization, an 8192x16 argmax — 0.003% of the FLOPs) runs on host
as part of the dispatch/sharding step; each core receives the tokens
routed to its two charts as transposed column blocks plus per-token gate
scales, with its two charts' residual weights resident in SBUF.

Residual decomposition: with gates g1+g2 = s, algebraically
    out = g1*W_c1 q + g2*W_c2 q = g1*(W_c1+I) q + g2*(W_c2+I) q - s*q.
The device computes the (W+I) matmuls in bf16 — since W ~= -I + 0.01*N,
the residual W+I is ~100x smaller than q's projection, so bf16 rounding
error scales with the residual, not the output (measured ~6e-4 scale-
relative absmax vs fp32 reference). Accumulation is fp32 in PSUM, gate
scaling on-device in fp32, and the exact -s*q correction is applied in
fp32 during the host unshard/combine step.

Matmul orientation: residual W^T chunks are the stationary operand and
token columns are the moving operand (out = Y^T tiles [u, tokens]), so
token capacities need no 128-alignment — per-slot capacity is exactly
the max chart load and the only padded work is capacity skew.

self-contained: only imports concourse (globally installed) + numpy.
"""

import numpy as np

P = 128
D = 1024  # DIM_Q == DIM_U
NUM_CHARTS = 16
TOP_K = 2
N_CORES = 8
MAX_MM = 512  # moving-operand / PSUM-bank limit per matmul

_NC_CACHE: dict = {}


def _blocks(cap0: int, cap1: int):
    """Static token-block list: (slot, col0, length) covering [0, cap0+cap1),
    slot-contiguous, each block <= MAX_MM. Greedy max-size blocks (bigger
    early blocks = more PE work per PSUM bank during the preload), with the
    tail rebalanced to avoid degenerate slivers."""
    out = []
    col = 0
    for s, cap in ((0, cap0), (1, cap1)):
        lens = []
        left = cap
        while left > 0:
            ln = min(MAX_MM, left)
            lens.append(ln)
            left -= ln
        if len(lens) >= 2 and lens[-1] < 64:
            merged = lens[-2] + lens[-1]
            lens[-2], lens[-1] = -(-merged // 2), merged // 2
        for ln in lens:
            out.append((s, col, ln))
            col += ln
    return out


def _build(cap0: int, cap1: int):
    """Build + compile the SPMD single-core program (same on all 8 cores).

    Inputs per core (CT = cap0 + cap1):
      xt  [8, 128, CT] bf16 : X^T q-chunked: xt[qc, p, col] =
                              q[token(col)][qc*128+p]. Slot 0 tokens in
                              columns [0, cap0), slot 1 after (zero-pad).
      wt  [2, 8, 128, 1024] bf16 : (W+I)^T of the 2 charts, q-chunked the
                              same way: wt[s, qc, p, u].
      scv [1, CT] f32 : gate scale of token(col); broadcast across
                              partitions on device via k=1 matmuls.
    Output:
      y [1024, CT] f32 : Y^T — gate-scaled residual chart outputs.
    """
    import concourse.mybir as mybir
    from concourse import bacc
    from concourse.tile import TileContext

    CT = cap0 + cap1
    f32 = mybir.dt.float32
    bf16 = mybir.dt.bfloat16
    blocks = _blocks(cap0, cap1)

    nc = bacc.Bacc("TRN2", target_bir_lowering=False, debug=False)
    xt = nc.dram_tensor("xt", [8, P, CT], bf16, kind="ExternalInput")
    wt = nc.dram_tensor("wt", [2, 8, P, D], bf16, kind="ExternalInput")
    scv = nc.dram_tensor("scv", [1, CT], f32, kind="ExternalInput")
    y = nc.dram_tensor("y", [D, CT], f32, kind="ExternalOutput")

    with TileContext(nc) as tc:
        with (
            tc.tile_pool(name="wpool", bufs=1) as wpool,
            tc.tile_pool(name="xpool", bufs=1) as xpool,
            tc.tile_pool(name="spool", bufs=1) as spool,
            tc.tile_pool(name="opool", bufs=4) as opool,
            tc.tile_pool(name="psum", bufs=8, space="PSUM") as psum,
        ):
            # gate-scale broadcast [1, CT] -> [128, CT] via k=1 matmuls with
            # a ones column; cheap, and warms the PE during the preload.
            # sc_row's DMA is issued first so the broadcast starts ASAP.
            sc_row = spool.tile([1, CT], f32, name="sc_row")
            nc.sync.dma_start(sc_row[:], scv[:])
            ones = spool.tile([1, P], f32, name="ones")
            nc.vector.memset(ones[:], 1.0)
            sc_sb = spool.tile([P, CT], f32, name="sc_sb")
            for bi, (_s, c0, ln) in enumerate(blocks):
                bps = psum.tile([P, ln], f32, tag="ps", name=f"bps_{bi}")
                nc.tensor.matmul(
                    bps[:],
                    lhsT=ones[:],
                    rhs=sc_row[:, c0 : c0 + ln],
                    start=True,
                    stop=True,
                )
                nc.vector.tensor_copy(sc_sb[:, c0 : c0 + ln], bps[:])

            # X^T resident BLOCK-major: one tile per token block holding all
            # 8 q-chunks side by side, loaded in a single DMA. Processing is
            # block-outer / u-inner from the start: block 0 alone provides
            # ~13.7us of accumulate work that chases the slot-0 W chunk
            # arrivals (PE saturates from ~2.5us), every (block, u) group is
            # completable the moment its inputs land, PSUM banks recycle per
            # group, and Y^T stores are small per-(block, u) DMAs spread
            # evenly across the run.
            nb = len(blocks)
            x_sb = [None] * nb
            w_sb = {0: [None] * 8, 1: [None] * 8}

            def _load_x(bi):
                _s, c0, ln = blocks[bi]
                t = xpool.tile([P, 8 * ln], bf16, tag=f"x_{bi}", name=f"x_{bi}")
                nc.sync.dma_start(
                    t[:].rearrange("p (qc t) -> p qc t", qc=8),
                    xt[:, :, c0 : c0 + ln].rearrange("qc p t -> p qc t"),
                )
                x_sb[bi] = t

            def _load_w(s, qc):
                t = wpool.tile([P, D], bf16, tag=f"w_{s}_{qc}", name=f"w_{s}_{qc}")
                nc.sync.dma_start(t[:], wt[s, qc])
                w_sb[s][qc] = t

            _load_w(0, 0)
            _load_x(0)
            for qc in range(1, 8):
                _load_w(0, qc)
                if qc == 3 and nb > 1:
                    _load_x(1)
                if qc == 7 and nb > 2:
                    _load_x(2)
            for qc in range(8):
                _load_w(1, qc)
            for bi in range(nb):
                if x_sb[bi] is None:
                    _load_x(bi)

            for bi, (s, c0, ln) in enumerate(blocks):
                for u in range(8):
                    ps = psum.tile([P, ln], f32, tag="ps", name=f"ps_{bi}_{u}")
                    for qc in range(8):
                        nc.tensor.matmul(
                            ps[:],
                            lhsT=w_sb[s][qc][:, u * P : (u + 1) * P],
                            rhs=x_sb[bi][:, qc * ln : (qc + 1) * ln],
                            start=(qc == 0),
                            stop=(qc == 7),
                        )
                    ob = opool.tile([P, ln], f32, tag="o", name=f"o_{bi}_{u}")
                    nc.vector.tensor_tensor(
                        out=ob[:],
                        in0=ps[:],
                        in1=sc_sb[:, c0 : c0 + ln],
                        op=mybir.AluOpType.mult,
                    )
                    # small spread-out Y^T store on the ACT queue
                    nc.scalar.dma_start(y[u * P : (u + 1) * P, c0 : c0 + ln], ob[:])
    nc.compile()
    return nc


def _get_nc(cap0: int, cap1: int):
    key = (cap0, cap1)
    if key not in _NC_CACHE:
        _NC_CACHE[key] = _build(cap0, cap1)
    return _NC_CACHE[key]


def _route(weights: np.ndarray):
    """Host router: top-2 charts + normalized gates (matches jax.lax.top_k
    tie-breaking: lower index first)."""
    b = weights.shape[0]
    ar = np.arange(b)
    i1 = np.argmax(weights, axis=1)
    v1 = weights[ar, i1]
    w2 = weights.copy()
    w2[ar, i1] = -np.inf
    i2 = np.argmax(w2, axis=1)
    v2 = w2[ar, i2]
    s = np.clip(v1 + v2, 1e-8, None)
    return i1, i2, v1 / s, v2 / s


def kernel(q, weights, W_stack):
    import ml_dtypes

    q = np.ascontiguousarray(np.asarray(q, dtype=np.float32))
    weights = np.ascontiguousarray(np.asarray(weights, dtype=np.float32))
    W_stack = np.asarray(W_stack, dtype=np.float32)

    B = q.shape[0]
    i1, i2, g1, g2 = _route(weights)

    # flatten the (token, chart) pairs and group by chart
    flat_chart = np.concatenate([i1, i2])
    flat_tok = np.concatenate([np.arange(B), np.arange(B)])
    flat_gate = np.concatenate([g1, g2]).astype(np.float32)
    order = np.argsort(flat_chart, kind="stable")
    counts = np.bincount(flat_chart, minlength=NUM_CHARTS)
    starts = np.zeros(NUM_CHARTS + 1, dtype=np.int64)
    starts[1:] = np.cumsum(counts)

    # balanced pairing: largest chart with smallest on the same core
    by_size = np.argsort(-counts, kind="stable")
    slot_chart = np.empty((N_CORES, 2), dtype=np.int64)
    for core in range(N_CORES):
        slot_chart[core, 0] = by_size[core]
        slot_chart[core, 1] = by_size[NUM_CHARTS - 1 - core]

    cap0 = int(counts[slot_chart[:, 0]].max())
    cap1 = int(counts[slot_chart[:, 1]].max())
    CT = cap0 + cap1

    # residual weights (W + I)^T per chart, bf16
    eye = np.eye(D, dtype=np.float32)
    Rt = (W_stack.transpose(0, 2, 1) + eye[None]).astype(ml_dtypes.bfloat16)
    q_bf = q.astype(ml_dtypes.bfloat16)

    in_maps = []
    pos = np.empty(2 * B, dtype=np.int64)  # pair j -> global Y^T column
    for core in range(N_CORES):
        xp = np.zeros((CT, D), dtype=ml_dtypes.bfloat16)  # packed tokens
        scv_c = np.zeros(CT, dtype=np.float32)
        wtm = np.empty((2, 8, P, D), dtype=ml_dtypes.bfloat16)
        for slot in range(2):
            c = int(slot_chart[core, slot])
            wtm[slot] = Rt[c].reshape(8, P, D)
            sel = order[starts[c] : starts[c + 1]]
            n = len(sel)
            col0 = slot * cap0
            xp[col0 : col0 + n] = q_bf[flat_tok[sel]]
            scv_c[col0 : col0 + n] = flat_gate[sel]
            pos[sel] = core * CT + col0 + np.arange(n)
        xtm = np.ascontiguousarray(xp.T).reshape(8, P, CT)
        in_maps.append({"xt": xtm, "wt": wtm, "scv": scv_c[None, :]})

    from concourse.bass_utils import run_bass_kernel_spmd

    nc = _get_nc(cap0, cap1)
    res = run_bass_kernel_spmd(nc, in_maps, core_ids=list(range(N_CORES)))
    # y is Y^T [1024, CT] per core; stack as [8*CT, 1024] token-major
    Yg = np.concatenate(
        [np.ascontiguousarray(np.asarray(res.results[i]["y"]).T) for i in range(N_CORES)],
        axis=0,
    )

    # exact combine: out = g1*Y1 + g2*Y2 - (g1+g2)*q   (Y are residual outs)
    gs = (g1 + g2).astype(np.float32)
    out = Yg[pos[:B]] + Yg[pos[B:]] - gs[:, None] * q
    return np.ascontiguousarray(out, dtype=np.float32)
